# revision 19
# baseline (speedup 1.0000x reference)
"""Trainium2 Bass kernel for nn_Decoder_51539607552479.

DecoderModule.forward: bilinear-upsample xt (32->64, align_corners) ->
xfuse = xup + alpha*xm -> conv3x3(512->512)+BN+ReLU -> conv3x3(512->256)
+BN+ReLU.  Pure data parallel: batch dim (8) across the 8 NeuronCores,
weights replicated.

Per-core device program (Tile/Bacc), fp8 DoubleRow matmuls:
 - conv3x3 via accumulating float8e4 matmuls in DoubleRow perf mode at
   0.5 cycles/row.  Precision via a scaled hi/lo decomposition: weights
   pre-scaled W = w*2^k into e4m3's normal range (2^-k folded into the
   BN drain scale) and split W = W_hi + W_lo; activations stored as
   interleaved fp8 pairs (x_hi = q8(x) at even bytes, x_lo = q8(x-x_hi)
   at odd bytes).  Per ci-tile and tap:
     A-tile: planes (W_hi, W_hi) x (x_hi, x_lo) = W_hi*x   [9 taps]
     B-tile: planes (W_lo[t], W_lo[t']) x (x_hi@t, x_hi@t') corrects
       W_lo*x_hi for two taps at once.  conv0 corrects 6 taps (3 B),
       conv1 corrects 8 (4 B); uncorrected-tap sets picked by a numpy
       scheme simulation (rel err ~0.0175 vs the 2e-2 gate).
 - matmul rhs APs are (plane, row, col) 3-free-dim so each matmul
   covers only the 64 real output columns per row (no pad column in
   the PE sweep); PSUM tiles are [128, nr*64] contiguous.
 - upsample runs in bf16 on DVE (2x mode) with a parity decomposition;
   alpha is folded into xm host-side (xm' = alpha*xm in bf16) and the
   y-pattern weights are pre-broadcast host-side so every mul/add is a
   packed-bf16 tensor_tensor.  hi = q8(stg) on Act; lo = q8(stg - hi)
   on Pool for quarter 0 planes 0/2, on DVE for the rest (keeps Pool's
   in-order queue free for conv0 drain los).
 - XI planes are zeroed pads-only (row 0, row 65+tail, col-0 strip).
 - DMA order interleaves weight slabs (w0 slab 0 split in half) with
   per-quarter inputs so the first conv matmul starts ~7us in; fp32r
   warmup matmuls bridge the PE from t~1.5us to first-conv so the
   p-state ramp is spent on throwaway work.
 - conv0 drain: Act reads PSUM twice (Relu+BN to an fp32 stage and to
   the fp8 hi plane), Pool writes the lo plane.
 - BN scale/shift precomputed on host, packed into one [128,12] tensor.
"""
import sys

if '/opt/trn_rl_repo' not in sys.path:
    sys.path.insert(0, '/opt/trn_rl_repo')

import numpy as np
import ml_dtypes
import concourse.bacc as bacc
import concourse.mybir as mybir
from concourse.ap import AP
from concourse.tile import TileContext
from concourse.bass_utils import run_bass_kernel_spmd

F32 = mybir.dt.float32
F32R = mybir.dt.float32r
BF16 = mybir.dt.bfloat16
F8 = mybir.dt.float8e4
ALU = mybir.AluOpType
ACTF = mybir.ActivationFunctionType
DR_MODE = mybir.MatmulPerfMode.DoubleRow
E4 = ml_dtypes.float8_e4m3
BF = ml_dtypes.bfloat16
EPS = 1e-5

S = 65                    # padded row stride (shared side pads)
PAD_LEN = 66 * 65 + 2     # 4292: 66 rows + tail (incl final garbage elem)
ROW_TILES = [(r, 6) for r in range(0, 60, 6)] + [(60, 4)]   # (r0, nrows)
N_CORES = 8
# W_lo correction tap pairs per layer (tap = dy*3+dx).  Chosen by numpy
# scheme sim: conv0 leaves taps {2,6,7} uncorrected, conv1 leaves {1}.
B_PAIRS = {0: [(0, 1), (3, 4), (5, 8)],
           1: [(0, 2), (3, 5), (6, 8), (4, 7)]}
N_TILES = {l: 9 + len(B_PAIRS[l]) for l in (0, 1)}
WCOLS = {l: 4 * N_TILES[l] * 256 for l in (0, 1)}
import os as _os
N_WARM = int(_os.environ.get("KWARM", "74"))
_FP = _os.environ.get(
    "KFILL", "0,0,0,0,0,0,0,0,0,0,0,0,0,0,0,0")  # (B,A)x4c for rg0,rg1 @q0


def _v2(ap2d, offset, rows, rowstep, cols):
    """[128, rows, cols] strided view of a [128, L] AP starting at offset."""
    sl = ap2d[:, offset: offset + rows * rowstep]
    return sl.rearrange("p (r c) -> p r c", c=rowstep)[:, :, 0:cols]


def _sv(tile, offset, dims):
    """Raw strided view of a tile: dims = [(stride, count), ...] free dims."""
    a = tile[:]
    return AP(a.tensor, a.offset + offset,
              [list(a.ap[0])] + [[s, c] for (s, c) in dims])


def build_patterns():
    k = np.arange(1, 32)
    ko = np.arange(0, 31)
    wxe = -(k / 63.0)            # even x: x[k] + wxe[k-1]*d[k-1]
    wxo = (31 - ko) / 63.0       # odd  x: x[k] + wxo[k]*d[k]
    wye = -(k / 63.0)            # even y
    wyo = (31 - ko) / 63.0       # odd  y
    return np.concatenate([wxe, wxo, wye, wyo]).astype(BF)


def build_nc():
    nc = bacc.Bacc(None, target_bir_lowering=True)

    xt_d = nc.dram_tensor("xt", [512, 1024], BF16, kind="ExternalInput")
    xm_d = nc.dram_tensor("xm", [512, 4096], BF16, kind="ExternalInput")
    pat_d = nc.dram_tensor("pat", [124], BF16, kind="ExternalInput")
    w0_d = nc.dram_tensor("w0f8", [4 * 128, WCOLS[0]], F8, kind="ExternalInput")
    w1_d = nc.dram_tensor("w1f8", [2 * 128, WCOLS[1]], F8, kind="ExternalInput")
    bn_d = nc.dram_tensor("bnall", [128, 12], F32, kind="ExternalInput")
    out_d = nc.dram_tensor("out", [256, 4096], F32, kind="ExternalOutput")

    with TileContext(nc) as tc:
        with tc.tile_pool(name="main", bufs=1) as P, \
             tc.tile_pool(name="wp0", bufs=4) as WP0, \
             tc.tile_pool(name="wp1", bufs=2) as WP1, \
             tc.tile_pool(name="xmp", bufs=2) as XMP, \
             tc.tile_pool(name="xtp", bufs=2) as XTP, \
             tc.tile_pool(name="stg", bufs=2) as STG, \
             tc.tile_pool(name="drp", bufs=6) as DRP, \
             tc.tile_pool(name="outp", bufs=3) as OUTP, \
             tc.tile_pool(name="psum", bufs=8, space="PSUM") as PS:

            # ---------------- DMA plan (program order = queue order) --------
            pat = P.tile([128, 124], BF16, tag="pat")
            nc.sync.dma_start(pat[:], pat_d[:].partition_broadcast(128))
            xt_q0 = []
            xm_q0 = [None] * 4
            xt_t0 = XTP.tile([128, 352], BF16, tag="xtq", name="xt00")
            nc.sync.dma_start(xt_t0[:, 0:9 * 32], xt_d[0:128, 0:9 * 32])
            xt_q0.append(xt_t0)
            xm_t0 = XMP.tile([128, 1088], BF16, tag="xm", name="xm00")
            nc.sync.dma_start(xm_t0[:, 0:1024], xm_d[0:128, 0:1024])
            xm_q0[0] = xm_t0

            w0sl = []
            wt = WP0.tile([128, WCOLS[0]], F8, tag="w", name="w0sl0")
            half = WCOLS[0] // 2
            nc.sync.dma_start(wt[:, 0:half], w0_d[0:128, 0:half])
            w0sl.append(wt)

            for ct in range(1, 4):
                t = XTP.tile([128, 352], BF16, tag=f"xtq0_{ct}", name=f"xt0{ct}")
                nc.sync.dma_start(t[:, 0:9 * 32],
                                  xt_d[ct * 128:(ct + 1) * 128, 0:9 * 32])
                xt_q0.append(t)
            t = XMP.tile([128, 1088], BF16, tag="xmq0_1", name="xm01")
            nc.sync.dma_start(t[:, 0:1024], xm_d[128:256, 0:1024])
            xm_q0[1] = t

            nc.sync.dma_start(wt[:, half:], w0_d[0:128, half:])

            for ct in range(2, 4):
                t = XMP.tile([128, 1088], BF16, tag=f"xmq0_{ct}", name=f"xm0{ct}")
                nc.sync.dma_start(t[:, 0:1024],
                                  xm_d[ct * 128:(ct + 1) * 128, 0:1024])
                xm_q0[ct] = t

            bnall = P.tile([128, 12], F32, tag="bnall")
            nc.sync.dma_start(bnall[:], bn_d[:, :])
            # col j of bnall: layer0 scale q (0..3), layer0 shift q (4..7),
            # layer1 scale q (8..9), layer1 shift q (10..11)
            bn_scale = {(0, q): bnall[:, q:q + 1] for q in range(4)}
            bn_shift = {(0, q): bnall[:, 4 + q:5 + q] for q in range(4)}
            for q in range(2):
                bn_scale[(1, q)] = bnall[:, 8 + q:9 + q]
                bn_shift[(1, q)] = bnall[:, 10 + q:11 + q]

            wt = WP0.tile([128, WCOLS[0]], F8, tag="w", name="w0sl1")
            nc.sync.dma_start(wt[:], w0_d[128:256, :])
            w0sl.append(wt)
            # w0sl2/3, w1 and later quarters' inputs are queued inside the
            # upsample loop below to interleave with per-quarter DMAs.

            # ---------------- engine-local init ----------------
            wscr = P.tile([128, 384], F32R, tag="wscr")
            nc.vector.memset(wscr[:].bitcast(F32), 0.0)

            # y-pattern rows expanded to 64 cols on Act (keeps DVE y-muls
            # in packed-bf16 2x mode); ct0-q0 uses the broadcast form since
            # it runs before the expansion completes
            pat_exp = P.tile([128, 62 * 64], BF16, tag="pat_exp")

            def expand_pat(r0_, r1_):
                nc.scalar.activation(
                    _v2(pat_exp, r0_ * 64, r1_ - r0_, 64, 64),
                    pat[:, 62 + r0_: 62 + r1_].unsqueeze(2)
                       .broadcast_to((128, r1_ - r0_, 64)),
                    ACTF.Copy)

            # only the rows units (0,4)/(4,8) need are expanded up front;
            # the rest queue on Act behind the startup-critical hi's
            expand_pat(0, 8)
            expand_pat(31, 39)

            # interleaved fp8 activation planes: byte 2p = hi, 2p+1 = lo
            XI0 = [P.tile([128, 2 * PAD_LEN], F8, tag=f"XI0_{c}",
                          name=f"XI0_{c}") for c in range(4)]
            XI1 = [P.tile([128, 2 * PAD_LEN], F8, tag=f"XI1_{c}",
                          name=f"XI1_{c}") for c in range(4)]

            def zero_pads(t_):
                # row 0 (incl col-0), row 65 + tail, col-0 strip rows 1..64
                nc.gpsimd.memset(t_[:, 0:2 * S], 0.0)
                nc.gpsimd.memset(t_[:, 2 * 65 * S:2 * PAD_LEN], 0.0)
                nc.gpsimd.memset(_sv(t_, 2 * S, [(2 * S, 64), (1, 2)]), 0.0)

            for c in range(4):
                zero_pads(XI0[c])

            # PE warmup: dummy fp32r matmuls bridge the p-state ramp while
            # the upsample prefix runs
            pw = PS.tile([128, 256], F32, tag="cpsum", name="pwarm")
            for _ in range(N_WARM):
                nc.tensor.matmul(pw[:], wscr[:, 0:128], wscr[:, 128:384],
                                 start=True, stop=True, skip_group_check=True)

            def hi_view(t_, pos0, rows, cols):
                return _sv(t_, 2 * pos0, [(2 * S, rows), (2, cols)])

            def lo_view(t_, pos0, rows, cols):
                return _sv(t_, 2 * pos0 + 1, [(2 * S, rows), (2, cols)])

            # ---------------- phase A: upsample + fuse + fp8 split ----------
            # out-row quarters [2*j0, 2*j0+16); per quarter per ci-tile the
            # x-interp is recomputed on just input rows [j0-1, j0+9).
            pending_lo = []          # deferred DVE lo ops

            for j0, j1 in ((0, 4), (4, 8), (8, 16), (16, 24), (24, 32)):
                jstart = max(j0 - 1, 0)
                jstop2 = min(j1 + 1, 32)       # xh rows [jstart, jstop2)
                ny = jstop2 - jstart
                nro = 2 * (j1 - j0)            # output rows this unit
                for ct in range(4):
                    eng = nc.vector
                    if j0 < 8:
                        xt_t = xt_q0[ct]
                        xm_sb = xm_q0[ct]
                        xmoff = 2 * j0 * 64
                        xtoff = jstart * 32      # tile row 0 = input row 0
                    else:
                        xmoff = 0
                        xtoff = 0
                        xt_t = XTP.tile([128, 352], BF16, tag="xtq")
                        nc.sync.dma_start(
                            xt_t[:, 0:ny * 32],
                            xt_d[ct * 128:(ct + 1) * 128,
                                 jstart * 32: jstart * 32 + ny * 32])
                        xm_sb = XMP.tile([128, 1088], BF16, tag="xm")
                        nc.sync.dma_start(
                            xm_sb[:, 0:1024],
                            xm_d[ct * 128:(ct + 1) * 128,
                                 2 * j0 * 64: 2 * j0 * 64 + 1024])
                    stg = STG.tile([128, 1088], BF16, tag="stg",
                                   name=f"stg{j0}_{ct}")
                    # --- x-interp on rows [jstart, jstop2) ---
                    d = P.tile([128, 10 * 31], BF16, tag="dtmp")
                    dv = d[:, 0:ny * 31].rearrange("p (y k) -> p y k", k=31)
                    eng.tensor_sub(dv,
                                   _v2(xt_t, xtoff + 1, ny, 32, 31),
                                   _v2(xt_t, xtoff, ny, 32, 31))
                    xh = P.tile([128, 10 * 64], BF16, tag="xh")
                    xh4 = xh[:, 0:ny * 64].rearrange("p (y k t) -> p y k t",
                                                     k=32, t=2)
                    tx = P.tile([128, 10 * 31], BF16, tag="ttmp")
                    txv = tx[:, 0:ny * 31].rearrange("p (y k) -> p y k", k=31)
                    # even cols 2k (k=1..31): xh = x[k] + wxe[k-1]*d[k-1]
                    pxe = pat[:, 0:31].unsqueeze(1).broadcast_to((128, ny, 31))
                    eng.tensor_mul(txv, dv, pxe)
                    eng.tensor_add(xh4[:, :, 1:32, 0:1].squeeze(),
                                   _v2(xt_t, xtoff + 1, ny, 32, 31),
                                   txv)
                    eng.tensor_copy(xh4[:, :, 0:1, 0:1].squeeze(),
                                    _v2(xt_t, xtoff, ny, 32, 1).squeeze())
                    # odd cols 2k+1 (k=0..30): xh = x[k] + wxo[k]*d[k]
                    pxo = pat[:, 31:62].unsqueeze(1).broadcast_to((128, ny, 31))
                    eng.tensor_mul(txv, dv, pxo)
                    eng.tensor_add(xh4[:, :, 0:31, 1:2].squeeze(),
                                   _v2(xt_t, xtoff, ny, 32, 31),
                                   txv)
                    eng.tensor_copy(xh4[:, :, 31:32, 1:2].squeeze(),
                                    _v2(xt_t, xtoff + 31, ny, 32, 1).squeeze())
                    # --- y-interp: D[j'] = xh[j'+1] - xh[j'] ---
                    nD = ny - 1                 # D rows [jstart, jstop2-1)
                    D = P.tile([128, 9 * 64], BF16, tag="Dtmp")
                    eng.tensor_sub(D[:, 0:nD * 64],
                                   xh[:, 64: 64 + nD * 64],
                                   xh[:, 0: nD * 64])
                    ty = P.tile([128, 8 * 64], BF16, tag="tytmp")
                    # even rows Y=2j, j in [max(j0,1), j1):
                    #   stg[2(j-j0)] = xh[j] + wye[j-1]*D[j-1] + xm'
                    jlo = max(j0, 1)
                    n = j1 - jlo
                    tyv = ty[:, 0:n * 64].rearrange("p (r c) -> p r c", c=64)
                    pye = _v2(pat_exp, (jlo - 1) * 64, n, 64, 64)
                    eng.tensor_mul(
                        tyv, _v2(D, (jlo - 1 - jstart) * 64, n, 64, 64), pye)
                    eng.tensor_add(
                        tyv,
                        _v2(xm_sb, xmoff + (2 * jlo - 2 * j0) * 64, n, 128, 64),
                        tyv)
                    eng.tensor_add(
                        _v2(stg, 2 * (jlo - j0) * 64, n, 128, 64),
                        _v2(xh, (jlo - jstart) * 64, n, 64, 64), tyv)
                    if j0 == 0:
                        eng.tensor_add(stg[:, 0:64], xm_sb[:, 0:64],
                                       xh[:, 0:64])
                    # odd rows Y=2j+1, j in [j0, min(j1,31)):
                    #   stg[2(j-j0)+1] = xh[j] + wyo[j]*D[j] + xm'
                    jhi = min(j1, 31)
                    n = jhi - j0
                    tyv = ty[:, 0:n * 64].rearrange("p (r c) -> p r c", c=64)
                    pyo = _v2(pat_exp, (31 + j0) * 64, n, 64, 64)
                    eng.tensor_mul(
                        tyv, _v2(D, (j0 - jstart) * 64, n, 64, 64), pyo)
                    eng.tensor_add(
                        tyv, _v2(xm_sb, xmoff + 64, n, 128, 64), tyv)
                    eng.tensor_add(
                        _v2(stg, 64, n, 128, 64),
                        _v2(xh, (j0 - jstart) * 64, n, 64, 64), tyv)
                    if j1 == 32:
                        eng.tensor_add(
                            stg[:, (nro - 1) * 64: nro * 64],
                            xm_sb[:, xmoff + (nro - 1) * 64:
                                  xmoff + nro * 64],
                            xh[:, (31 - jstart) * 64: (31 - jstart) * 64 + 64])
                    # --- fp8 split: hi = q8(stg) on Act, lo = q8(stg-hi) ---
                    stgv = stg[:, 0:nro * 64].rearrange("p (r c) -> p r c",
                                                        c=64)
                    pos0 = (2 * j0 + 1) * S + 1
                    hv = hi_view(XI0[ct], pos0, nro, 64)
                    lv = lo_view(XI0[ct], pos0, nro, 64)
                    nc.scalar.activation(hv, stgv, ACTF.Copy)
                    if j0 < 8:
                        if ct == 3:
                            nc.vector.tensor_sub(lv, stgv, hv)
                        else:
                            nc.gpsimd.tensor_sub(lv, stgv, hv)
                    else:
                        # defer to DVE, emitted after the NEXT plane's chain
                        # so DVE doesn't idle waiting for Act's hi
                        pending_lo.append((stgv, hv, lv))
                    if len(pending_lo) > 1:
                        s_, h_, l_ = pending_lo.pop(0)
                        nc.vector.tensor_sub(l_, s_, h_)
                if j0 == 4:
                    for r0_ in (8, 14, 20, 26, 39, 45, 51, 57):
                        expand_pat(r0_, min(r0_ + 6, 62) if r0_ >= 39
                                   else min(r0_ + 6, 31))
                    # XI1 pads + trailing weight DMAs slot in behind q0
                    for c in range(4):
                        zero_pads(XI1[c])
                    for sl in (2, 3):
                        wt = WP0.tile([128, WCOLS[0]], F8, tag="w",
                                      name=f"w0sl{sl}")
                        nc.sync.dma_start(
                            wt[:], w0_d[sl * 128:(sl + 1) * 128, :])
                        w0sl.append(wt)
                if j0 == 16:
                    w1sl = []
                    for sl in range(2):
                        wt = WP1.tile([128, WCOLS[1]], F8, tag="w1",
                                      name=f"w1sl{sl}")
                        nc.sync.dma_start(
                            wt[:], w1_d[sl * 128:(sl + 1) * 128, :])
                        w1sl.append(wt)
            while pending_lo:
                s_, h_, l_ = pending_lo.pop(0)
                nc.vector.tensor_sub(l_, s_, h_)

            # ---------------- conv layers ----------------
            def filler(n):
                for _ in range(n):
                    nc.tensor.matmul(pw[:], wscr[:, 0:128], wscr[:, 128:384],
                                     start=True, stop=True,
                                     skip_group_check=True)

            def conv_layer(layer, n_cot, XI, wsl, drain, rgs, fillers):
                n_tiles = N_TILES[layer]
                ti_order = list(range(9, n_tiles)) + list(range(9))
                for rgi, rg in enumerate(rgs):
                    for q in range(n_cot):
                        ptiles = []
                        for rt in rg:
                            r0, nr = ROW_TILES[rt]
                            pt = PS.tile([128, nr * 64], F32, tag="cpsum",
                                         name=f"ps_{layer}_{q}_{rt}")
                            ptiles.append(pt)
                        for c in range(4):
                            nB, nA = fillers.get((rgi, q, c), (0, 0))
                            for ti in ti_order:
                                if ti == 9:
                                    filler(nB)
                                elif ti == 0:
                                    filler(nA)
                                woff = (c * n_tiles + ti) * 256
                                lhsT = wsl[q][:, woff:woff + 256] \
                                    .rearrange("p (two m) -> p two m", m=128)
                                if ti < 9:
                                    dy, dx = divmod(ti, 3)
                                    ps_ = 1                 # (hi, lo) planes
                                else:
                                    t, t2 = B_PAIRS[layer][ti - 9]
                                    dy, dx = divmod(t, 3)
                                    dy2, dx2 = divmod(t2, 3)
                                    ps_ = 2 * ((dy2 - dy) * S + dx2 - dx)
                                for i, rt in enumerate(rg):
                                    r0, nr = ROW_TILES[rt]
                                    off = (r0 + dy) * S + dx
                                    rhs = _sv(XI[c], 2 * off,
                                              [(ps_, 2), (2 * S, nr), (2, 64)])
                                    nc.tensor.matmul(
                                        ptiles[i][:, 0:nr * 64],
                                        lhsT, rhs,
                                        start=(c == 0 and ti == ti_order[0]),
                                        stop=(c == 3 and ti == 8),
                                        perf_mode=DR_MODE,
                                        skip_group_check=True)
                        for i, rt in enumerate(rg):
                            drain(q, rt, ptiles[i])

            def drain0(q, rt, ptile):
                r0, nr = ROW_TILES[rt]
                pv = ptile[:, 0:nr * 64]
                ys = DRP.tile([128, 384], F32, tag="ys", name=f"ys{q}_{rt}")
                nc.scalar.activation(ys[:, 0:nr * 64], pv, ACTF.Relu,
                                     bias=bn_shift[(0, q)],
                                     scale=bn_scale[(0, q)])
                pos0 = (r0 + 1) * S + 1
                hv = hi_view(XI1[q], pos0, nr, 64)
                pvv = pv.rearrange("p (r c) -> p r c", c=64)
                nc.scalar.activation(hv, pvv, ACTF.Relu,
                                     bias=bn_shift[(0, q)],
                                     scale=bn_scale[(0, q)])
                lv = lo_view(XI1[q], pos0, nr, 64)
                ysv = ys[:, 0:nr * 64].rearrange("p (r c) -> p r c", c=64)
                nc.gpsimd.tensor_sub(lv, ysv, hv)

            def drain1(q, rt, ptile):
                r0, nr = ROW_TILES[rt]
                ob = OUTP.tile([128, 384], F32, tag="ob",
                               name=f"ob{q}_{rt}")
                pv = ptile[:, 0:nr * 64]
                nc.scalar.activation(ob[:, 0:nr * 64], pv, ACTF.Relu,
                                     bias=bn_shift[(1, q)],
                                     scale=bn_scale[(1, q)])
                nc.sync.dma_start(
                    out_d[q * 128:(q + 1) * 128,
                          r0 * 64:(r0 + nr) * 64],
                    ob[:, 0:nr * 64])

            def _fillmap():
                v = [int(x) for x in _FP.split(",")]
                m = {}
                for rgi in (0, 1):
                    for c in range(4):
                        i = rgi * 8 + c * 2
                        m[(rgi, 0, c)] = (v[i], v[i + 1])
                return m

            conv_layer(0, 4, XI0, w0sl, drain0,
                       [(0,), (1,), (2, 3, 4), (5, 6), (7, 8, 9, 10)],
                       _fillmap())
            conv_layer(1, 2, XI1, w1sl, drain1,
                       [(0, 1, 2, 3), (4, 5, 6, 7), (8, 9), (10,)], {})

    nc.finalize()
    return nc


_CACHED_NC = None


def _get_nc():
    global _CACHED_NC
    if _CACHED_NC is None:
        _CACHED_NC = build_nc()
    return _CACHED_NC


def _q8(a):
    return np.asarray(a, np.float32).astype(E4)


def _pack_layer(w, n_q, layer):
    """w: [CO, 512, 3, 3] f32 -> ([n_q*128, WCOLS[layer]] e4m3 slab, k).

    Slab rows: (q, ci_in); cols: (ci_tile, tile, plane, co) where tiles
    0-8 are A-tiles (W_hi, W_hi) per tap and 9+ are B-tiles
    (W_lo[t], W_lo[t']) per B_PAIRS entry."""
    k = float(2.0 ** np.floor(np.log2(128.0 / np.abs(w).max())))
    W = (w * k).astype(np.float32)
    Whi = _q8(W)
    Wlo = _q8(W - Whi.astype(np.float32))
    # [q, co, c, ci_in, tap] uint8
    hi = Whi.view(np.uint8).reshape(n_q, 128, 4, 128, 9)
    lo = Wlo.view(np.uint8).reshape(n_q, 128, 4, 128, 9)
    hi = hi.transpose(0, 3, 2, 4, 1)    # [q, ci_in, c, tap, co]
    lo = lo.transpose(0, 3, 2, 4, 1)
    tiles = []
    for tap in range(9):
        tiles.append(np.stack([hi[..., tap, :], hi[..., tap, :]], axis=-2))
    for t, t2 in B_PAIRS[layer]:
        tiles.append(np.stack([lo[..., t, :], lo[..., t2, :]], axis=-2))
    # [q, ci_in, c, n_tiles, 2, co]
    arr = np.stack(tiles, axis=3)
    slab = np.ascontiguousarray(arr).reshape(n_q * 128, WCOLS[layer])
    return slab.view(E4), k


def kernel(**inputs) -> np.ndarray:
    xt = np.ascontiguousarray(np.asarray(inputs["xt"], np.float32))   # [8,512,32,32]
    xm = np.ascontiguousarray(np.asarray(inputs["xm"], np.float32))   # [8,512,64,64]
    alpha = float(np.asarray(inputs["alpha"], np.float32).reshape(1)[0])
    w0 = np.asarray(inputs["w0"], np.float32)                         # [512,512,3,3]
    w1 = np.asarray(inputs["w1"], np.float32)                         # [256,512,3,3]

    w0f8, k0 = _pack_layer(w0, 4, 0)
    w1f8, k1 = _pack_layer(w1, 2, 1)
    pat = build_patterns()

    def bn(g, b, m, v, k):
        inv = np.asarray(g, np.float32) / np.sqrt(np.asarray(v, np.float32) + EPS)
        scale = (inv / k).astype(np.float32)
        shift = (np.asarray(b, np.float32)
                 - np.asarray(m, np.float32) * inv).astype(np.float32)
        return scale, shift

    bn0s, bn0h = bn(inputs["g0"], inputs["b0"], inputs["m0"], inputs["v0"], k0)
    bn1s, bn1h = bn(inputs["g1"], inputs["b1"], inputs["m1"], inputs["v1"], k1)
    bnall = np.zeros((128, 12), np.float32)
    for q in range(4):
        bnall[:, q] = bn0s[q * 128:(q + 1) * 128]
        bnall[:, 4 + q] = bn0h[q * 128:(q + 1) * 128]
    for q in range(2):
        bnall[:, 8 + q] = bn1s[q * 128:(q + 1) * 128]
        bnall[:, 10 + q] = bn1h[q * 128:(q + 1) * 128]

    xt_b = xt.astype(BF)
    xm_b = (alpha * xm).astype(BF)     # alpha folded host-side

    common = {"pat": pat, "w0f8": w0f8, "w1f8": w1f8, "bnall": bnall}

    in_maps = []
    for b in range(N_CORES):
        m = dict(common)
        m["xt"] = np.ascontiguousarray(xt_b[b].reshape(512, 1024))
        m["xm"] = np.ascontiguousarray(xm_b[b].reshape(512, 4096))
        in_maps.append(m)

    nc = _get_nc()
    res = run_bass_kernel_spmd(nc, in_maps, core_ids=list(range(N_CORES)))
    out = np.stack([res.results[b]["out"].reshape(256, 64, 64)
                    for b in range(N_CORES)], axis=0)
    return out.astype(np.float32)


# revision 20
# speedup vs baseline: 1.0011x; 1.0011x over previous
"""Trainium2 Bass kernel for nn_Decoder_51539607552479.

DecoderModule.forward: bilinear-upsample xt (32->64, align_corners) ->
xfuse = xup + alpha*xm -> conv3x3(512->512)+BN+ReLU -> conv3x3(512->256)
+BN+ReLU.  Pure data parallel: batch dim (8) across the 8 NeuronCores,
weights replicated.

Per-core device program (Tile/Bacc), fp8 DoubleRow matmuls:
 - conv3x3 via accumulating float8e4 matmuls in DoubleRow perf mode at
   0.5 cycles/row.  Precision via a scaled hi/lo decomposition: weights
   pre-scaled W = w*2^k into e4m3's normal range (2^-k folded into the
   BN drain scale) and split W = W_hi + W_lo; activations stored as
   interleaved fp8 pairs (x_hi = q8(x) at even bytes, x_lo = q8(x-x_hi)
   at odd bytes).  Per ci-tile and tap:
     A-tile: planes (W_hi, W_hi) x (x_hi, x_lo) = W_hi*x   [9 taps]
     B-tile: planes (W_lo[t], W_lo[t']) x (x_hi@t, x_hi@t') corrects
       W_lo*x_hi for two taps at once.  conv0 corrects 6 taps (3 B),
       conv1 corrects 8 (4 B); uncorrected-tap sets picked by a numpy
       scheme simulation (rel err ~0.0175 vs the 2e-2 gate).
 - matmul rhs APs are (plane, row, col) 3-free-dim so each matmul
   covers only the 64 real output columns per row (no pad column in
   the PE sweep); PSUM tiles are [128, nr*64] contiguous.
 - upsample runs in bf16 on DVE (2x mode) with a parity decomposition;
   alpha is folded into xm host-side (xm' = alpha*xm in bf16) and the
   y-pattern weights are pre-broadcast host-side so every mul/add is a
   packed-bf16 tensor_tensor.  hi = q8(stg) on Act; lo = q8(stg - hi)
   on Pool for quarter 0 planes 0/2, on DVE for the rest (keeps Pool's
   in-order queue free for conv0 drain los).
 - XI planes are zeroed pads-only (row 0, row 65+tail, col-0 strip).
 - DMA order interleaves weight slabs (w0 slab 0 split in half) with
   per-quarter inputs so the first conv matmul starts ~7us in; fp32r
   warmup matmuls bridge the PE from t~1.5us to first-conv so the
   p-state ramp is spent on throwaway work.
 - conv0 drain: Act reads PSUM twice (Relu+BN to an fp32 stage and to
   the fp8 hi plane), Pool writes the lo plane.
 - BN scale/shift precomputed on host, packed into one [128,12] tensor.
"""
import sys

if '/opt/trn_rl_repo' not in sys.path:
    sys.path.insert(0, '/opt/trn_rl_repo')

import numpy as np
import ml_dtypes
import concourse.bacc as bacc
import concourse.mybir as mybir
from concourse.ap import AP
from concourse.tile import TileContext
from concourse.bass_utils import run_bass_kernel_spmd

F32 = mybir.dt.float32
F32R = mybir.dt.float32r
BF16 = mybir.dt.bfloat16
F8 = mybir.dt.float8e4
ALU = mybir.AluOpType
ACTF = mybir.ActivationFunctionType
DR_MODE = mybir.MatmulPerfMode.DoubleRow
E4 = ml_dtypes.float8_e4m3
BF = ml_dtypes.bfloat16
EPS = 1e-5

S = 65                    # padded row stride (shared side pads)
PAD_LEN = 66 * 65 + 2     # 4292: 66 rows + tail (incl final garbage elem)
ROW_TILES = [(r, 6) for r in range(0, 60, 6)] + [(60, 4)]   # (r0, nrows)
N_CORES = 8
# W_lo correction tap pairs per layer (tap = dy*3+dx).  Chosen by numpy
# scheme sim: conv0 leaves taps {2,6,7} uncorrected, conv1 leaves {1}.
B_PAIRS = {0: [(0, 1), (3, 4), (5, 8)],
           1: [(0, 2), (3, 5), (6, 8), (4, 7)]}
N_TILES = {l: 9 + len(B_PAIRS[l]) for l in (0, 1)}
WCOLS = {l: 4 * N_TILES[l] * 256 for l in (0, 1)}
import os as _os
N_WARM = int(_os.environ.get("KWARM", "74"))
_FP = _os.environ.get(
    "KFILL", "0,0,0,0,0,0,0,0,0,0,0,0,0,0,0,0")  # (B,A)x4c for rg0,rg1 @q0


def _v2(ap2d, offset, rows, rowstep, cols):
    """[128, rows, cols] strided view of a [128, L] AP starting at offset."""
    sl = ap2d[:, offset: offset + rows * rowstep]
    return sl.rearrange("p (r c) -> p r c", c=rowstep)[:, :, 0:cols]


def _sv(tile, offset, dims):
    """Raw strided view of a tile: dims = [(stride, count), ...] free dims."""
    a = tile[:]
    return AP(a.tensor, a.offset + offset,
              [list(a.ap[0])] + [[s, c] for (s, c) in dims])


def build_patterns():
    k = np.arange(1, 32)
    ko = np.arange(0, 31)
    wxe = -(k / 63.0)            # even x: x[k] + wxe[k-1]*d[k-1]
    wxo = (31 - ko) / 63.0       # odd  x: x[k] + wxo[k]*d[k]
    wye = -(k / 63.0)            # even y
    wyo = (31 - ko) / 63.0       # odd  y
    return np.concatenate([wxe, wxo, wye, wyo]).astype(BF)


def build_nc():
    nc = bacc.Bacc(None, target_bir_lowering=True)

    xt_d = nc.dram_tensor("xt", [512, 1024], BF16, kind="ExternalInput")
    xm_d = nc.dram_tensor("xm", [512, 4096], BF16, kind="ExternalInput")
    pat_d = nc.dram_tensor("pat", [124], BF16, kind="ExternalInput")
    w0_d = nc.dram_tensor("w0f8", [4 * 128, WCOLS[0]], F8, kind="ExternalInput")
    w1_d = nc.dram_tensor("w1f8", [2 * 128, WCOLS[1]], F8, kind="ExternalInput")
    bn_d = nc.dram_tensor("bnall", [128, 12], F32, kind="ExternalInput")
    out_d = nc.dram_tensor("out", [256, 4096], F32, kind="ExternalOutput")

    with TileContext(nc) as tc:
        with tc.tile_pool(name="main", bufs=1) as P, \
             tc.tile_pool(name="wp0", bufs=4) as WP0, \
             tc.tile_pool(name="wp1", bufs=2) as WP1, \
             tc.tile_pool(name="xmp", bufs=2) as XMP, \
             tc.tile_pool(name="xtp", bufs=2) as XTP, \
             tc.tile_pool(name="stg", bufs=2) as STG, \
             tc.tile_pool(name="drp", bufs=6) as DRP, \
             tc.tile_pool(name="outp", bufs=3) as OUTP, \
             tc.tile_pool(name="psum", bufs=8, space="PSUM") as PS:

            # ---------------- DMA plan (program order = queue order) --------
            pat = P.tile([128, 124], BF16, tag="pat")
            nc.sync.dma_start(pat[:], pat_d[:].partition_broadcast(128))
            xt_q0 = []
            xm_q0 = [None] * 4
            xt_t0 = XTP.tile([128, 352], BF16, tag="xtq", name="xt00")
            nc.sync.dma_start(xt_t0[:, 0:9 * 32], xt_d[0:128, 0:9 * 32])
            xt_q0.append(xt_t0)
            xm_t0 = XMP.tile([128, 1088], BF16, tag="xm", name="xm00")
            nc.sync.dma_start(xm_t0[:, 0:1024], xm_d[0:128, 0:1024])
            xm_q0[0] = xm_t0

            w0sl = []
            wt = WP0.tile([128, WCOLS[0]], F8, tag="w", name="w0sl0")
            half = WCOLS[0] // 2
            nc.sync.dma_start(wt[:, 0:half], w0_d[0:128, 0:half])
            w0sl.append(wt)

            for ct in range(1, 4):
                t = XTP.tile([128, 352], BF16, tag=f"xtq0_{ct}", name=f"xt0{ct}")
                nc.sync.dma_start(t[:, 0:9 * 32],
                                  xt_d[ct * 128:(ct + 1) * 128, 0:9 * 32])
                xt_q0.append(t)
            t = XMP.tile([128, 1088], BF16, tag="xmq0_1", name="xm01")
            nc.sync.dma_start(t[:, 0:1024], xm_d[128:256, 0:1024])
            xm_q0[1] = t

            nc.sync.dma_start(wt[:, half:], w0_d[0:128, half:])

            for ct in range(2, 4):
                t = XMP.tile([128, 1088], BF16, tag=f"xmq0_{ct}", name=f"xm0{ct}")
                nc.sync.dma_start(t[:, 0:1024],
                                  xm_d[ct * 128:(ct + 1) * 128, 0:1024])
                xm_q0[ct] = t

            bnall = P.tile([128, 12], F32, tag="bnall")
            nc.sync.dma_start(bnall[:], bn_d[:, :])
            # col j of bnall: layer0 scale q (0..3), layer0 shift q (4..7),
            # layer1 scale q (8..9), layer1 shift q (10..11)
            bn_scale = {(0, q): bnall[:, q:q + 1] for q in range(4)}
            bn_shift = {(0, q): bnall[:, 4 + q:5 + q] for q in range(4)}
            for q in range(2):
                bn_scale[(1, q)] = bnall[:, 8 + q:9 + q]
                bn_shift[(1, q)] = bnall[:, 10 + q:11 + q]

            wt = WP0.tile([128, WCOLS[0]], F8, tag="w", name="w0sl1")
            nc.sync.dma_start(wt[:], w0_d[128:256, :])
            w0sl.append(wt)
            # w0sl2/3, w1 and later quarters' inputs are queued inside the
            # upsample loop below to interleave with per-quarter DMAs.

            # ---------------- engine-local init ----------------
            wscr = P.tile([128, 384], F32R, tag="wscr")
            nc.vector.memset(wscr[:].bitcast(F32), 0.0)

            # y-pattern rows expanded to 64 cols on Act (keeps DVE y-muls
            # in packed-bf16 2x mode); ct0-q0 uses the broadcast form since
            # it runs before the expansion completes
            pat_exp = P.tile([128, 62 * 64], BF16, tag="pat_exp")

            def expand_pat(r0_, r1_):
                nc.scalar.activation(
                    _v2(pat_exp, r0_ * 64, r1_ - r0_, 64, 64),
                    pat[:, 62 + r0_: 62 + r1_].unsqueeze(2)
                       .broadcast_to((128, r1_ - r0_, 64)),
                    ACTF.Copy)

            # only the rows units (0,4)/(4,8) need are expanded up front;
            # the rest queue on Act behind the startup-critical hi's
            expand_pat(0, 8)
            expand_pat(31, 39)

            # interleaved fp8 activation planes: byte 2p = hi, 2p+1 = lo
            XI0 = [P.tile([128, 2 * PAD_LEN], F8, tag=f"XI0_{c}",
                          name=f"XI0_{c}") for c in range(4)]
            XI1 = [P.tile([128, 2 * PAD_LEN], F8, tag=f"XI1_{c}",
                          name=f"XI1_{c}") for c in range(4)]

            def zero_pads(t_):
                # row 0 (incl col-0), row 65 + tail, col-0 strip rows 1..64
                nc.gpsimd.memset(t_[:, 0:2 * S], 0.0)
                nc.gpsimd.memset(t_[:, 2 * 65 * S:2 * PAD_LEN], 0.0)
                nc.gpsimd.memset(_sv(t_, 2 * S, [(2 * S, 64), (1, 2)]), 0.0)

            for c in range(4):
                zero_pads(XI0[c])

            # PE warmup: dummy fp32r matmuls bridge the p-state ramp while
            # the upsample prefix runs
            pw = PS.tile([128, 256], F32, tag="cpsum", name="pwarm")
            for _ in range(N_WARM):
                nc.tensor.matmul(pw[:], wscr[:, 0:128], wscr[:, 128:384],
                                 start=True, stop=True, skip_group_check=True)

            def hi_view(t_, pos0, rows, cols):
                return _sv(t_, 2 * pos0, [(2 * S, rows), (2, cols)])

            def lo_view(t_, pos0, rows, cols):
                return _sv(t_, 2 * pos0 + 1, [(2 * S, rows), (2, cols)])

            # ---------------- phase A: upsample + fuse + fp8 split ----------
            # out-row quarters [2*j0, 2*j0+16); per quarter per ci-tile the
            # x-interp is recomputed on just input rows [j0-1, j0+9).
            pending_lo = []          # deferred DVE lo ops

            for j0, j1 in ((0, 4), (4, 8), (8, 16), (16, 24), (24, 32)):
                jstart = max(j0 - 1, 0)
                jstop2 = min(j1 + 1, 32)       # xh rows [jstart, jstop2)
                ny = jstop2 - jstart
                nro = 2 * (j1 - j0)            # output rows this unit
                for ct in range(4):
                    eng = nc.vector
                    if j0 < 8:
                        xt_t = xt_q0[ct]
                        xm_sb = xm_q0[ct]
                        xmoff = 2 * j0 * 64
                        xtoff = jstart * 32      # tile row 0 = input row 0
                    else:
                        xmoff = 0
                        xtoff = 0
                        xt_t = XTP.tile([128, 352], BF16, tag="xtq")
                        nc.sync.dma_start(
                            xt_t[:, 0:ny * 32],
                            xt_d[ct * 128:(ct + 1) * 128,
                                 jstart * 32: jstart * 32 + ny * 32])
                        xm_sb = XMP.tile([128, 1088], BF16, tag="xm")
                        nc.sync.dma_start(
                            xm_sb[:, 0:1024],
                            xm_d[ct * 128:(ct + 1) * 128,
                                 2 * j0 * 64: 2 * j0 * 64 + 1024])
                    stg = STG.tile([128, 1088], BF16, tag="stg",
                                   name=f"stg{j0}_{ct}")
                    # --- x-interp on rows [jstart, jstop2) ---
                    d = P.tile([128, 10 * 31], BF16, tag="dtmp")
                    dv = d[:, 0:ny * 31].rearrange("p (y k) -> p y k", k=31)
                    eng.tensor_sub(dv,
                                   _v2(xt_t, xtoff + 1, ny, 32, 31),
                                   _v2(xt_t, xtoff, ny, 32, 31))
                    xh = P.tile([128, 10 * 64], BF16, tag="xh")
                    xh4 = xh[:, 0:ny * 64].rearrange("p (y k t) -> p y k t",
                                                     k=32, t=2)
                    tx = P.tile([128, 10 * 31], BF16, tag="ttmp")
                    txv = tx[:, 0:ny * 31].rearrange("p (y k) -> p y k", k=31)
                    # even cols 2k (k=1..31): xh = x[k] + wxe[k-1]*d[k-1]
                    pxe = pat[:, 0:31].unsqueeze(1).broadcast_to((128, ny, 31))
                    eng.tensor_mul(txv, dv, pxe)
                    eng.tensor_add(xh4[:, :, 1:32, 0:1].squeeze(),
                                   _v2(xt_t, xtoff + 1, ny, 32, 31),
                                   txv)
                    eng.tensor_copy(xh4[:, :, 0:1, 0:1].squeeze(),
                                    _v2(xt_t, xtoff, ny, 32, 1).squeeze())
                    # odd cols 2k+1 (k=0..30): xh = x[k] + wxo[k]*d[k]
                    pxo = pat[:, 31:62].unsqueeze(1).broadcast_to((128, ny, 31))
                    eng.tensor_mul(txv, dv, pxo)
                    eng.tensor_add(xh4[:, :, 0:31, 1:2].squeeze(),
                                   _v2(xt_t, xtoff, ny, 32, 31),
                                   txv)
                    eng.tensor_copy(xh4[:, :, 31:32, 1:2].squeeze(),
                                    _v2(xt_t, xtoff + 31, ny, 32, 1).squeeze())
                    # --- y-interp: D[j'] = xh[j'+1] - xh[j'] ---
                    nD = ny - 1                 # D rows [jstart, jstop2-1)
                    D = P.tile([128, 9 * 64], BF16, tag="Dtmp")
                    eng.tensor_sub(D[:, 0:nD * 64],
                                   xh[:, 64: 64 + nD * 64],
                                   xh[:, 0: nD * 64])
                    ty = P.tile([128, 8 * 64], BF16, tag="tytmp")
                    # even rows Y=2j, j in [max(j0,1), j1):
                    #   stg[2(j-j0)] = xh[j] + wye[j-1]*D[j-1] + xm'
                    jlo = max(j0, 1)
                    n = j1 - jlo
                    tyv = ty[:, 0:n * 64].rearrange("p (r c) -> p r c", c=64)
                    pye = _v2(pat_exp, (jlo - 1) * 64, n, 64, 64)
                    eng.tensor_mul(
                        tyv, _v2(D, (jlo - 1 - jstart) * 64, n, 64, 64), pye)
                    eng.tensor_add(
                        tyv,
                        _v2(xm_sb, xmoff + (2 * jlo - 2 * j0) * 64, n, 128, 64),
                        tyv)
                    eng.tensor_add(
                        _v2(stg, 2 * (jlo - j0) * 64, n, 128, 64),
                        _v2(xh, (jlo - jstart) * 64, n, 64, 64), tyv)
                    if j0 == 0:
                        eng.tensor_add(stg[:, 0:64], xm_sb[:, 0:64],
                                       xh[:, 0:64])
                    # odd rows Y=2j+1, j in [j0, min(j1,31)):
                    #   stg[2(j-j0)+1] = xh[j] + wyo[j]*D[j] + xm'
                    jhi = min(j1, 31)
                    n = jhi - j0
                    tyv = ty[:, 0:n * 64].rearrange("p (r c) -> p r c", c=64)
                    pyo = _v2(pat_exp, (31 + j0) * 64, n, 64, 64)
                    eng.tensor_mul(
                        tyv, _v2(D, (j0 - jstart) * 64, n, 64, 64), pyo)
                    eng.tensor_add(
                        tyv, _v2(xm_sb, xmoff + 64, n, 128, 64), tyv)
                    eng.tensor_add(
                        _v2(stg, 64, n, 128, 64),
                        _v2(xh, (j0 - jstart) * 64, n, 64, 64), tyv)
                    if j1 == 32:
                        eng.tensor_add(
                            stg[:, (nro - 1) * 64: nro * 64],
                            xm_sb[:, xmoff + (nro - 1) * 64:
                                  xmoff + nro * 64],
                            xh[:, (31 - jstart) * 64: (31 - jstart) * 64 + 64])
                    # --- fp8 split: hi = q8(stg) on Act, lo = q8(stg-hi) ---
                    stgv = stg[:, 0:nro * 64].rearrange("p (r c) -> p r c",
                                                        c=64)
                    pos0 = (2 * j0 + 1) * S + 1
                    hv = hi_view(XI0[ct], pos0, nro, 64)
                    lv = lo_view(XI0[ct], pos0, nro, 64)
                    nc.scalar.activation(hv, stgv, ACTF.Copy)
                    if j0 < 8:
                        if ct == 3:
                            nc.vector.tensor_sub(lv, stgv, hv)
                        else:
                            nc.gpsimd.tensor_sub(lv, stgv, hv)
                    else:
                        # defer to DVE, emitted after the NEXT plane's chain
                        # so DVE doesn't idle waiting for Act's hi
                        pending_lo.append((stgv, hv, lv))
                    if len(pending_lo) > 1:
                        s_, h_, l_ = pending_lo.pop(0)
                        nc.vector.tensor_sub(l_, s_, h_)
                if j0 == 4:
                    for r0_ in (8, 14, 20, 26, 39, 45, 51, 57):
                        expand_pat(r0_, min(r0_ + 6, 62) if r0_ >= 39
                                   else min(r0_ + 6, 31))
                    # XI1 pads + trailing weight DMAs slot in behind q0
                    for c in range(4):
                        zero_pads(XI1[c])
                    for sl in (2, 3):
                        wt = WP0.tile([128, WCOLS[0]], F8, tag="w",
                                      name=f"w0sl{sl}")
                        nc.sync.dma_start(
                            wt[:], w0_d[sl * 128:(sl + 1) * 128, :])
                        w0sl.append(wt)
                if j0 == 16:
                    w1sl = []
                    for sl in range(2):
                        wt = WP1.tile([128, WCOLS[1]], F8, tag="w1",
                                      name=f"w1sl{sl}")
                        nc.sync.dma_start(
                            wt[:], w1_d[sl * 128:(sl + 1) * 128, :])
                        w1sl.append(wt)
            while pending_lo:
                s_, h_, l_ = pending_lo.pop(0)
                nc.vector.tensor_sub(l_, s_, h_)

            # ---------------- conv layers ----------------
            def filler(n):
                for _ in range(n):
                    nc.tensor.matmul(pw[:], wscr[:, 0:128], wscr[:, 128:384],
                                     start=True, stop=True,
                                     skip_group_check=True)

            def conv_layer(layer, n_cot, XI, wsl, drain, rgs, fillers):
                n_tiles = N_TILES[layer]
                ti_order = list(range(9, n_tiles)) + list(range(9))
                for rgi, rg in enumerate(rgs):
                    if layer == 0 and rgi == 0:
                        # defer (q0,c3)/(q1,c3) past (q1,c0-c2): moves work
                        # ahead of the startup-critical ct3 plane dependency
                        qc_sched = [(0, (0, 1, 2)), (1, (0, 1, 2)),
                                    (0, (3,)), (1, (3,)),
                                    (2, (0, 1, 2, 3)), (3, (0, 1, 2, 3))]
                    else:
                        qc_sched = [(q, (0, 1, 2, 3)) for q in range(n_cot)]
                    qtiles = {}
                    for q, cs in qc_sched:
                        if q not in qtiles:
                            qtiles[q] = []
                            for rt in rg:
                                r0, nr = ROW_TILES[rt]
                                pt = PS.tile([128, nr * 64], F32, tag="cpsum",
                                             name=f"ps_{layer}_{q}_{rt}")
                                qtiles[q].append(pt)
                        ptiles = qtiles[q]
                        for c in cs:
                            nB, nA = fillers.get((rgi, q, c), (0, 0))
                            for ti in ti_order:
                                if ti == 9:
                                    filler(nB)
                                elif ti == 0:
                                    filler(nA)
                                woff = (c * n_tiles + ti) * 256
                                lhsT = wsl[q][:, woff:woff + 256] \
                                    .rearrange("p (two m) -> p two m", m=128)
                                if ti < 9:
                                    dy, dx = divmod(ti, 3)
                                    ps_ = 1                 # (hi, lo) planes
                                else:
                                    t, t2 = B_PAIRS[layer][ti - 9]
                                    dy, dx = divmod(t, 3)
                                    dy2, dx2 = divmod(t2, 3)
                                    ps_ = 2 * ((dy2 - dy) * S + dx2 - dx)
                                for i, rt in enumerate(rg):
                                    r0, nr = ROW_TILES[rt]
                                    off = (r0 + dy) * S + dx
                                    rhs = _sv(XI[c], 2 * off,
                                              [(ps_, 2), (2 * S, nr), (2, 64)])
                                    nc.tensor.matmul(
                                        ptiles[i][:, 0:nr * 64],
                                        lhsT, rhs,
                                        start=(c == 0 and ti == ti_order[0]),
                                        stop=(c == 3 and ti == 8),
                                        perf_mode=DR_MODE,
                                        skip_group_check=True)
                        if 3 in cs:
                            for i, rt in enumerate(rg):
                                drain(q, rt, ptiles[i])

            def drain0(q, rt, ptile):
                r0, nr = ROW_TILES[rt]
                pv = ptile[:, 0:nr * 64]
                ys = DRP.tile([128, 384], F32, tag="ys", name=f"ys{q}_{rt}")
                nc.scalar.activation(ys[:, 0:nr * 64], pv, ACTF.Relu,
                                     bias=bn_shift[(0, q)],
                                     scale=bn_scale[(0, q)])
                pos0 = (r0 + 1) * S + 1
                hv = hi_view(XI1[q], pos0, nr, 64)
                pvv = pv.rearrange("p (r c) -> p r c", c=64)
                nc.scalar.activation(hv, pvv, ACTF.Relu,
                                     bias=bn_shift[(0, q)],
                                     scale=bn_scale[(0, q)])
                lv = lo_view(XI1[q], pos0, nr, 64)
                ysv = ys[:, 0:nr * 64].rearrange("p (r c) -> p r c", c=64)
                nc.gpsimd.tensor_sub(lv, ysv, hv)

            def drain1(q, rt, ptile):
                r0, nr = ROW_TILES[rt]
                ob = OUTP.tile([128, 384], F32, tag="ob",
                               name=f"ob{q}_{rt}")
                pv = ptile[:, 0:nr * 64]
                nc.scalar.activation(ob[:, 0:nr * 64], pv, ACTF.Relu,
                                     bias=bn_shift[(1, q)],
                                     scale=bn_scale[(1, q)])
                nc.sync.dma_start(
                    out_d[q * 128:(q + 1) * 128,
                          r0 * 64:(r0 + nr) * 64],
                    ob[:, 0:nr * 64])

            def _fillmap():
                v = [int(x) for x in _FP.split(",")]
                m = {}
                for rgi in (0, 1):
                    for c in range(4):
                        i = rgi * 8 + c * 2
                        m[(rgi, 0, c)] = (v[i], v[i + 1])
                return m

            conv_layer(0, 4, XI0, w0sl, drain0,
                       [(0,), (1,), (2, 3, 4), (5, 6), (7, 8, 9, 10)],
                       _fillmap())
            conv_layer(1, 2, XI1, w1sl, drain1,
                       [(0, 1, 2, 3), (4, 5, 6, 7), (8, 9), (10,)], {})

    nc.finalize()
    return nc


_CACHED_NC = None


def _get_nc():
    global _CACHED_NC
    if _CACHED_NC is None:
        _CACHED_NC = build_nc()
    return _CACHED_NC


def _q8(a):
    return np.asarray(a, np.float32).astype(E4)


def _pack_layer(w, n_q, layer):
    """w: [CO, 512, 3, 3] f32 -> ([n_q*128, WCOLS[layer]] e4m3 slab, k).

    Slab rows: (q, ci_in); cols: (ci_tile, tile, plane, co) where tiles
    0-8 are A-tiles (W_hi, W_hi) per tap and 9+ are B-tiles
    (W_lo[t], W_lo[t']) per B_PAIRS entry."""
    k = float(2.0 ** np.floor(np.log2(128.0 / np.abs(w).max())))
    W = (w * k).astype(np.float32)
    Whi = _q8(W)
    Wlo = _q8(W - Whi.astype(np.float32))
    # [q, co, c, ci_in, tap] uint8
    hi = Whi.view(np.uint8).reshape(n_q, 128, 4, 128, 9)
    lo = Wlo.view(np.uint8).reshape(n_q, 128, 4, 128, 9)
    hi = hi.transpose(0, 3, 2, 4, 1)    # [q, ci_in, c, tap, co]
    lo = lo.transpose(0, 3, 2, 4, 1)
    tiles = []
    for tap in range(9):
        tiles.append(np.stack([hi[..., tap, :], hi[..., tap, :]], axis=-2))
    for t, t2 in B_PAIRS[layer]:
        tiles.append(np.stack([lo[..., t, :], lo[..., t2, :]], axis=-2))
    # [q, ci_in, c, n_tiles, 2, co]
    arr = np.stack(tiles, axis=3)
    slab = np.ascontiguousarray(arr).reshape(n_q * 128, WCOLS[layer])
    return slab.view(E4), k


def kernel(**inputs) -> np.ndarray:
    xt = np.ascontiguousarray(np.asarray(inputs["xt"], np.float32))   # [8,512,32,32]
    xm = np.ascontiguousarray(np.asarray(inputs["xm"], np.float32))   # [8,512,64,64]
    alpha = float(np.asarray(inputs["alpha"], np.float32).reshape(1)[0])
    w0 = np.asarray(inputs["w0"], np.float32)                         # [512,512,3,3]
    w1 = np.asarray(inputs["w1"], np.float32)                         # [256,512,3,3]

    w0f8, k0 = _pack_layer(w0, 4, 0)
    w1f8, k1 = _pack_layer(w1, 2, 1)
    pat = build_patterns()

    def bn(g, b, m, v, k):
        inv = np.asarray(g, np.float32) / np.sqrt(np.asarray(v, np.float32) + EPS)
        scale = (inv / k).astype(np.float32)
        shift = (np.asarray(b, np.float32)
                 - np.asarray(m, np.float32) * inv).astype(np.float32)
        return scale, shift

    bn0s, bn0h = bn(inputs["g0"], inputs["b0"], inputs["m0"], inputs["v0"], k0)
    bn1s, bn1h = bn(inputs["g1"], inputs["b1"], inputs["m1"], inputs["v1"], k1)
    bnall = np.zeros((128, 12), np.float32)
    for q in range(4):
        bnall[:, q] = bn0s[q * 128:(q + 1) * 128]
        bnall[:, 4 + q] = bn0h[q * 128:(q + 1) * 128]
    for q in range(2):
        bnall[:, 8 + q] = bn1s[q * 128:(q + 1) * 128]
        bnall[:, 10 + q] = bn1h[q * 128:(q + 1) * 128]

    xt_b = xt.astype(BF)
    xm_b = (alpha * xm).astype(BF)     # alpha folded host-side

    common = {"pat": pat, "w0f8": w0f8, "w1f8": w1f8, "bnall": bnall}

    in_maps = []
    for b in range(N_CORES):
        m = dict(common)
        m["xt"] = np.ascontiguousarray(xt_b[b].reshape(512, 1024))
        m["xm"] = np.ascontiguousarray(xm_b[b].reshape(512, 4096))
        in_maps.append(m)

    nc = _get_nc()
    res = run_bass_kernel_spmd(nc, in_maps, core_ids=list(range(N_CORES)))
    out = np.stack([res.results[b]["out"].reshape(256, 64, 64)
                    for b in range(N_CORES)], axis=0)
    return out.astype(np.float32)
